# revision 6
# baseline (speedup 1.0000x reference)
"""MinGRU cell kernel for Trainium2, 8 NeuronCores, data-parallel over batch.

Reference computation (per batch b):
    z = x @ Wz.T + bz ; g = sigmoid(z)          [T, H]
    u = x @ Wh.T + bh                            [T, H]
    h_t = (1 - g_t) * h_{t-1} + g_t * u_t        scan over T
Output hs [B, T, H].

Per-core plan (core b handles batch b, B == 8 == n_cores):
  - PE-transpose x [T, D] -> xT [D, T] (128x128 blocks through PSUM)
  - z/u matmuls with weight lhsT (WzT/WhT, PE-transposed on device),
    rhs = xT chunks, out in [H-part, T-free] layout
  - a = sigmoid(-z - bz) = 1 - g  (ScalarE, fused scale/bias)
    g = 1 - a                     (GPSIMD tensor_scalar)
    b = (u + bh) * g              (VectorE scalar_tensor_tensor)
  - h = a * h_prev + b as hardware scan along free dim
    (VectorE tensor_tensor_scan, chunk-chained carry)
  - PE-transpose hs [H, T] -> [T, H], contiguous DMA out
"""

import sys

sys.path.insert(0, "/opt/trn_rl_repo")

from contextlib import ExitStack

import numpy as np

import bass_rust
import concourse.bass as bass
import concourse.mybir as mybir
import concourse.tile as tile
from concourse.bass_utils import run_bass_kernel_spmd
from concourse.masks import make_identity

B, T, D, H = 8, 4096, 256, 256
P = 128
TC = 512          # t-chunk (PSUM bank = 512 fp32)
NCH = T // TC     # 8 chunks
NB = TC // P      # 4 t-blocks per chunk
F32 = mybir.dt.float32
F32R = mybir.dt.float32r
AOP = mybir.AluOpType

# knobs
MM_DT = "f32r"    # "f32" (4 cyc/row, exact) or "f32r" (1 cyc/row, relaxed)
N_CORES = 8


def _split_sync_waits(nc, max_waits=1):
    """walrus CoreV3 here accepts at most 1 sync-wait command per
    instruction; move excess waits onto preceding same-engine NoOps."""
    n = 0
    cnt = [0]
    for f in nc.m.functions:
        for bb in f.blocks:
            out = []
            changed = False
            for inst in bb.instructions:
                si = inst.sync_info
                if si is not None and si.on_wait and len(si.on_wait) > max_waits:
                    waits = list(si.on_wait)
                    extra, keep = waits[:-max_waits], waits[-max_waits:]
                    for j in range(0, len(extra), max_waits):
                        cnt[0] += 1
                        nop = bass_rust.InstNoOp(
                            name=f"I-waitsplit-{cnt[0]}", engine=inst.engine
                        )
                        nop.sync_info = mybir.SyncInfo(
                            on_wait=extra[j : j + max_waits], on_update=[]
                        )
                        out.append(nop)
                    inst.sync_info = mybir.SyncInfo(
                        on_wait=keep, on_update=list(si.on_update or [])
                    )
                    changed = True
                    n += 1
                out.append(inst)
            if changed:
                bb.instructions = out
    return n


def _mm_dt():
    return F32R if MM_DT == "f32r" else F32


def build_nc():
    nc = bass.Bass()
    x = nc.dram_tensor("x", [T, D], F32, kind="ExternalInput")
    h0 = nc.dram_tensor("h0", [H], F32, kind="ExternalInput")
    Wz = nc.dram_tensor("Wz", [H, D], F32, kind="ExternalInput")
    bz = nc.dram_tensor("bz", [H], F32, kind="ExternalInput")
    Wh = nc.dram_tensor("Wh", [H, D], F32, kind="ExternalInput")
    bh = nc.dram_tensor("bh", [H], F32, kind="ExternalInput")
    out = nc.dram_tensor("out", [T, H], F32, kind="ExternalOutput")

    with tile.TileContext(nc) as tc, ExitStack() as ctx:
        consts = ctx.enter_context(tc.tile_pool(name="consts", bufs=1))
        xnat_p = ctx.enter_context(tc.tile_pool(name="xnat", bufs=3))
        xt_p = ctx.enter_context(tc.tile_pool(name="xt", bufs=4))
        gates_p = ctx.enter_context(tc.tile_pool(name="gates", bufs=3))
        hs_p = ctx.enter_context(tc.tile_pool(name="hs", bufs=4))
        outb_p = ctx.enter_context(tc.tile_pool(name="outb", bufs=2))
        xtps_p = ctx.enter_context(tc.tile_pool(name="xtps", bufs=2, space="PSUM"))
        zu_p = ctx.enter_context(tc.tile_pool(name="zu", bufs=4, space="PSUM"))
        ops_p = ctx.enter_context(tc.tile_pool(name="ops", bufs=2, space="PSUM"))

        ident = consts.tile([P, P], F32)
        make_identity(nc, ident)

        # --- biases / h0 as [128, 2] (col = h half) ---
        def load_cols(name, dram):
            t = consts.tile([P, 2], F32, tag=name)
            nc.gpsimd.dma_start(out=t, in_=dram[:].rearrange("(n p) -> p n", p=P))
            return t

        bz_sb = load_cols("bz_sb", bz)
        bh_sb = load_cols("bh_sb", bh)
        h0_sb = load_cols("h0_sb", h0)
        nbz_sb = consts.tile([P, 2], F32)
        nc.vector.tensor_scalar_mul(nbz_sb, bz_sb, -1.0)

        # --- weights: load [H, D] natural, PE-transpose to WT [D-part, H] ---
        def load_wt(name, dram):
            nat = xnat_p.tile([P, 2, D], F32, tag="x_nat")
            nc.sync.dma_start(out=nat, in_=dram[:, :].rearrange("(n p) d -> p n d", p=P))
            tiles = []
            for kk in range(2):
                ps = xtps_p.tile([P, TC], F32, tag="xT_ps")
                for hh in range(2):
                    nc.tensor.transpose(
                        ps[:, hh * P : (hh + 1) * P],
                        nat[:, hh, kk * P : (kk + 1) * P],
                        ident,
                    )
                sb = consts.tile([P, H], _mm_dt(), tag=f"{name}_{kk}")
                nc.scalar.copy(sb, ps[:, 0:H])
                tiles.append(sb)
            return tiles

        WzT = load_wt("WzT", Wz)
        WhT = load_wt("WhT", Wh)

        hs_prev = [None, None]
        for c in range(NCH):
            t0 = c * TC
            # 1) load x chunk [512, 256] as [128, 4, 256]
            x_nat = xnat_p.tile([P, NB, D], F32, tag="x_nat")
            nc.sync.dma_start(
                out=x_nat,
                in_=x[t0 : t0 + TC, :].rearrange("(n p) d -> p n d", p=P),
            )
            # 2) PE transpose -> xT [128(d), 512(t)] per k-half
            xT = []
            for kk in range(2):
                ps = xtps_p.tile([P, TC], F32, tag="xT_ps")
                for n in range(NB):
                    nc.tensor.transpose(
                        ps[:, n * P : (n + 1) * P],
                        x_nat[:, n, kk * P : (kk + 1) * P],
                        ident,
                    )
                sb = xt_p.tile([P, TC], _mm_dt(), tag="xT_sb")
                nc.scalar.copy(sb, ps)
                xT.append(sb)
            # 3) matmuls + gates + scan per h-half
            hs_cur = [None, None]
            for hh in range(2):
                z_ps = zu_p.tile([P, TC], F32, tag="zu_ps")
                u_ps = zu_p.tile([P, TC], F32, tag="zu_ps")
                for kk in range(2):
                    nc.tensor.matmul(
                        z_ps,
                        WzT[kk][:, hh * P : (hh + 1) * P],
                        xT[kk],
                        start=(kk == 0),
                        stop=(kk == 1),
                    )
                for kk in range(2):
                    nc.tensor.matmul(
                        u_ps,
                        WhT[kk][:, hh * P : (hh + 1) * P],
                        xT[kk],
                        start=(kk == 0),
                        stop=(kk == 1),
                    )
                # a = sigmoid(-z - bz) = 1 - g
                a_sb = gates_p.tile([P, TC], F32, tag="a_sb")
                nc.scalar.activation(
                    a_sb, z_ps, mybir.ActivationFunctionType.Sigmoid,
                    bias=nbz_sb[:, hh : hh + 1], scale=-1.0,
                )
                # g = 1 - a
                g_sb = gates_p.tile([P, TC], F32, tag="g_sb")
                nc.gpsimd.tensor_scalar(g_sb, a_sb, -1.0, 1.0, AOP.mult, AOP.add)
                # b = (u + bh) * g
                b_sb = gates_p.tile([P, TC], F32, tag="b_sb")
                nc.vector.scalar_tensor_tensor(
                    b_sb, u_ps, bh_sb[:, hh : hh + 1], g_sb, AOP.add, AOP.mult
                )
                # scan: h[t] = a[t] * h[t-1] + b[t]
                hs = hs_p.tile([P, TC], F32, tag="hs")
                init = (
                    h0_sb[:, hh : hh + 1]
                    if c == 0
                    else hs_prev[hh][:, TC - 1 : TC]
                )
                nc.vector.tensor_tensor_scan(hs, a_sb, b_sb, init, AOP.mult, AOP.add)
                hs_cur[hh] = hs
            hs_prev = hs_cur
            # 4) transpose hs [H, T] -> [T, H] and store
            out_sb = outb_p.tile([P, NB * H], F32, tag="out_sb")
            for q in range(2):
                ps = ops_p.tile([P, TC], F32, tag="out_ps")
                for dn in range(2):
                    for hh in range(2):
                        n = 2 * q + dn
                        nc.tensor.transpose(
                            ps[:, dn * H + hh * P : dn * H + (hh + 1) * P],
                            hs_cur[hh][:, n * P : (n + 1) * P],
                            ident,
                        )
                eng = nc.scalar if q == 0 else nc.vector
                if q == 0:
                    nc.scalar.copy(out_sb[:, q * TC : (q + 1) * TC], ps)
                else:
                    nc.vector.tensor_copy(out_sb[:, q * TC : (q + 1) * TC], ps)
            nc.sync.dma_start(
                out=out[t0 : t0 + TC, :].rearrange("(n p) h -> p n h", p=P),
                in_=out_sb.rearrange("p (n h) -> p n h", n=NB),
            )

    _split_sync_waits(nc)
    return nc


_NC_CACHE = None


def _get_nc():
    global _NC_CACHE
    if _NC_CACHE is None:
        _NC_CACHE = build_nc()
    return _NC_CACHE


def kernel(x, h0, Wz, bz, Wh, bh):
    x = np.ascontiguousarray(np.asarray(x, dtype=np.float32))
    h0 = np.ascontiguousarray(np.asarray(h0, dtype=np.float32))
    Wz = np.ascontiguousarray(np.asarray(Wz, dtype=np.float32))
    bz = np.ascontiguousarray(np.asarray(bz, dtype=np.float32))
    Wh = np.ascontiguousarray(np.asarray(Wh, dtype=np.float32))
    bh = np.ascontiguousarray(np.asarray(bh, dtype=np.float32))
    nc = _get_nc()
    in_maps = [
        {"x": x[b], "h0": h0[b], "Wz": Wz, "bz": bz, "Wh": Wh, "bh": bh}
        for b in range(N_CORES)
    ]
    res = run_bass_kernel_spmd(nc, in_maps, list(range(N_CORES))).results
    return np.stack([res[b]["out"] for b in range(N_CORES)], axis=0)


# revision 9
# speedup vs baseline: 830.8422x; 830.8422x over previous
"""MinGRU cell kernel for Trainium2, 8 NeuronCores, data-parallel over batch.

Reference computation (per batch b):
    z = x @ Wz.T + bz ; g = sigmoid(z)          [T, H]
    u = x @ Wh.T + bh                            [T, H]
    h_t = (1 - g_t) * h_{t-1} + g_t * u_t        scan over T
Output hs [B, T, H].

Per-core plan (core b handles batch b, B == 8 == n_cores):
  - PE-transpose x [T, D] -> xT [D, T] (128x128 blocks through PSUM)
  - z/u matmuls with weight lhsT (WzT/WhT, PE-transposed on device),
    rhs = xT chunks, out in [H-part, T-free] layout
  - a = sigmoid(-z - bz) = 1 - g  (ScalarE, fused scale/bias)
    g = 1 - a                     (GPSIMD tensor_scalar)
    b = (u + bh) * g              (VectorE scalar_tensor_tensor)
  - h = a * h_prev + b as hardware scan along free dim
    (VectorE tensor_tensor_scan, chunk-chained carry)
  - PE-transpose hs [H, T] -> [T, H], contiguous DMA out
"""

import sys

sys.path.insert(0, "/opt/trn_rl_repo")

from contextlib import ExitStack

import numpy as np

import bass_rust
import concourse.bass as bass
import concourse.mybir as mybir
import concourse.tile as tile
from concourse.bass_utils import run_bass_kernel_spmd
from concourse.masks import make_identity

B, T, D, H = 8, 4096, 256, 256
P = 128
TC = 512          # t-chunk (PSUM bank = 512 fp32)
NCH = T // TC     # 8 chunks
NB = TC // P      # 4 t-blocks per chunk
F32 = mybir.dt.float32
F32R = mybir.dt.float32r
AOP = mybir.AluOpType

# knobs
MM_DT = "f32r"    # "f32" (4 cyc/row, exact) or "f32r" (1 cyc/row, relaxed)
N_CORES = 8


def _split_sync_waits(nc, max_waits=1):
    """walrus CoreV3 here accepts at most 1 sync-wait command per
    instruction; move excess waits onto preceding same-engine NoOps."""
    n = 0
    cnt = [0]
    for f in nc.m.functions:
        for bb in f.blocks:
            out = []
            changed = False
            for inst in bb.instructions:
                si = inst.sync_info
                if si is not None and si.on_wait and len(si.on_wait) > max_waits:
                    waits = list(si.on_wait)
                    extra, keep = waits[:-max_waits], waits[-max_waits:]
                    for j in range(0, len(extra), max_waits):
                        cnt[0] += 1
                        nop = bass_rust.InstNoOp(
                            name=f"I-waitsplit-{cnt[0]}", engine=inst.engine
                        )
                        nop.sync_info = mybir.SyncInfo(
                            on_wait=extra[j : j + max_waits], on_update=[]
                        )
                        out.append(nop)
                    inst.sync_info = mybir.SyncInfo(
                        on_wait=keep, on_update=list(si.on_update or [])
                    )
                    changed = True
                    n += 1
                out.append(inst)
            if changed:
                bb.instructions = out
    return n


def _mm_dt():
    return F32R if MM_DT == "f32r" else F32


def build_nc(reps=1):
    nc = bass.Bass()
    x = nc.dram_tensor("x", [T, D], F32, kind="ExternalInput")
    h0 = nc.dram_tensor("h0", [H], F32, kind="ExternalInput")
    Wz = nc.dram_tensor("Wz", [H, D], F32, kind="ExternalInput")
    bz = nc.dram_tensor("bz", [H], F32, kind="ExternalInput")
    Wh = nc.dram_tensor("Wh", [H, D], F32, kind="ExternalInput")
    bh = nc.dram_tensor("bh", [H], F32, kind="ExternalInput")
    out = nc.dram_tensor("out", [T, H], F32, kind="ExternalOutput")

    with tile.TileContext(nc) as tc, ExitStack() as ctx:
        consts = ctx.enter_context(tc.tile_pool(name="consts", bufs=1))
        xnat_p = ctx.enter_context(tc.tile_pool(name="xnat", bufs=3))
        xt_p = ctx.enter_context(tc.tile_pool(name="xt", bufs=4))
        gates_p = ctx.enter_context(tc.tile_pool(name="gates", bufs=3))
        hs_p = ctx.enter_context(tc.tile_pool(name="hs", bufs=4))
        outb_p = ctx.enter_context(tc.tile_pool(name="outb", bufs=2))
        xtps_p = ctx.enter_context(tc.tile_pool(name="xtps", bufs=2, space="PSUM"))
        zu_p = ctx.enter_context(tc.tile_pool(name="zu", bufs=4, space="PSUM"))
        ops_p = ctx.enter_context(tc.tile_pool(name="ops", bufs=2, space="PSUM"))

        for _rep in range(reps):
            _emit_body(
                nc, tc, consts, xnat_p, xt_p, gates_p, hs_p, outb_p,
                xtps_p, zu_p, ops_p, x, h0, Wz, bz, Wh, bh, out,
            )

    _split_sync_waits(nc)
    return nc


def _emit_body(
    nc, tc, consts, xnat_p, xt_p, gates_p, hs_p, outb_p,
    xtps_p, zu_p, ops_p, x, h0, Wz, bz, Wh, bh, out,
):
    if True:
        ident = consts.tile([P, P], F32)
        make_identity(nc, ident)

        # --- biases / h0 as [128, 2] (col = h half) ---
        def load_cols(name, dram):
            t = consts.tile([P, 2], F32, tag=name)
            nc.gpsimd.dma_start(out=t, in_=dram[:].rearrange("(n p) -> p n", p=P))
            return t

        bz_sb = load_cols("bz_sb", bz)
        bh_sb = load_cols("bh_sb", bh)
        h0_sb = load_cols("h0_sb", h0)
        nbz_sb = consts.tile([P, 2], F32)
        nc.vector.tensor_scalar_mul(nbz_sb, bz_sb, -1.0)

        # --- weights: load [H, D] natural, PE-transpose to WT [D-part, H] ---
        def load_wt(name, dram):
            nat = xnat_p.tile([P, 2, D], F32, tag="x_nat")
            nc.sync.dma_start(out=nat, in_=dram[:, :].rearrange("(n p) d -> p n d", p=P))
            tiles = []
            for kk in range(2):
                ps = xtps_p.tile([P, TC], F32, tag="xT_ps")
                for hh in range(2):
                    nc.tensor.transpose(
                        ps[:, hh * P : (hh + 1) * P],
                        nat[:, hh, kk * P : (kk + 1) * P],
                        ident,
                    )
                sb = consts.tile([P, H], _mm_dt(), tag=f"{name}_{kk}")
                nc.scalar.copy(sb, ps[:, 0:H])
                tiles.append(sb)
            return tiles

        WzT = load_wt("WzT", Wz)
        WhT = load_wt("WhT", Wh)

        hs_prev = [None, None]
        for c in range(NCH):
            t0 = c * TC
            # 1) load x chunk [512, 256] as [128, 4, 256]
            x_nat = xnat_p.tile([P, NB, D], F32, tag="x_nat")
            nc.sync.dma_start(
                out=x_nat,
                in_=x[t0 : t0 + TC, :].rearrange("(n p) d -> p n d", p=P),
            )
            # 2) PE transpose -> xT [128(d), 512(t)] per k-half
            xT = []
            for kk in range(2):
                ps = xtps_p.tile([P, TC], F32, tag="xT_ps")
                for n in range(NB):
                    nc.tensor.transpose(
                        ps[:, n * P : (n + 1) * P],
                        x_nat[:, n, kk * P : (kk + 1) * P],
                        ident,
                    )
                sb = xt_p.tile([P, TC], _mm_dt(), tag="xT_sb")
                nc.scalar.copy(sb, ps)
                xT.append(sb)
            # 3) matmuls + gates + scan per h-half
            hs_cur = [None, None]
            for hh in range(2):
                z_ps = zu_p.tile([P, TC], F32, tag="zu_ps")
                u_ps = zu_p.tile([P, TC], F32, tag="zu_ps")
                for kk in range(2):
                    nc.tensor.matmul(
                        z_ps,
                        WzT[kk][:, hh * P : (hh + 1) * P],
                        xT[kk],
                        start=(kk == 0),
                        stop=(kk == 1),
                    )
                for kk in range(2):
                    nc.tensor.matmul(
                        u_ps,
                        WhT[kk][:, hh * P : (hh + 1) * P],
                        xT[kk],
                        start=(kk == 0),
                        stop=(kk == 1),
                    )
                # a = sigmoid(-z - bz) = 1 - g
                a_sb = gates_p.tile([P, TC], F32, tag="a_sb")
                nc.scalar.activation(
                    a_sb, z_ps, mybir.ActivationFunctionType.Sigmoid,
                    bias=nbz_sb[:, hh : hh + 1], scale=-1.0,
                )
                # g = 1 - a
                g_sb = gates_p.tile([P, TC], F32, tag="g_sb")
                nc.gpsimd.tensor_scalar(g_sb, a_sb, -1.0, 1.0, AOP.mult, AOP.add)
                # b = (u + bh) * g
                b_sb = gates_p.tile([P, TC], F32, tag="b_sb")
                nc.vector.scalar_tensor_tensor(
                    b_sb, u_ps, bh_sb[:, hh : hh + 1], g_sb, AOP.add, AOP.mult
                )
                # scan: h[t] = a[t] * h[t-1] + b[t]
                hs = hs_p.tile([P, TC], F32, tag="hs")
                init = (
                    h0_sb[:, hh : hh + 1]
                    if c == 0
                    else hs_prev[hh][:, TC - 1 : TC]
                )
                nc.vector.tensor_tensor_scan(hs, a_sb, b_sb, init, AOP.mult, AOP.add)
                hs_cur[hh] = hs
            hs_prev = hs_cur
            # 4) transpose hs [H, T] -> [T, H] and store
            out_sb = outb_p.tile([P, NB * H], F32, tag="out_sb")
            for q in range(2):
                ps = ops_p.tile([P, TC], F32, tag="out_ps")
                for dn in range(2):
                    for hh in range(2):
                        n = 2 * q + dn
                        nc.tensor.transpose(
                            ps[:, dn * H + hh * P : dn * H + (hh + 1) * P],
                            hs_cur[hh][:, n * P : (n + 1) * P],
                            ident,
                        )
                eng = nc.scalar if q == 0 else nc.vector
                if q == 0:
                    nc.scalar.copy(out_sb[:, q * TC : (q + 1) * TC], ps)
                else:
                    nc.vector.tensor_copy(out_sb[:, q * TC : (q + 1) * TC], ps)
            nc.sync.dma_start(
                out=out[t0 : t0 + TC, :].rearrange("(n p) h -> p n h", p=P),
                in_=out_sb.rearrange("p (n h) -> p n h", n=NB),
            )


_NC_CACHE = {}


def _get_nc(reps=1):
    if reps not in _NC_CACHE:
        _NC_CACHE[reps] = build_nc(reps)
    return _NC_CACHE[reps]


def kernel(x, h0, Wz, bz, Wh, bh):
    x = np.ascontiguousarray(np.asarray(x, dtype=np.float32))
    h0 = np.ascontiguousarray(np.asarray(h0, dtype=np.float32))
    Wz = np.ascontiguousarray(np.asarray(Wz, dtype=np.float32))
    bz = np.ascontiguousarray(np.asarray(bz, dtype=np.float32))
    Wh = np.ascontiguousarray(np.asarray(Wh, dtype=np.float32))
    bh = np.ascontiguousarray(np.asarray(bh, dtype=np.float32))
    nc = _get_nc(1)
    in_maps = [
        {"x": x[b], "h0": h0[b], "Wz": Wz, "bz": bz, "Wh": Wh, "bh": bh}
        for b in range(N_CORES)
    ]
    res = run_bass_kernel_spmd(nc, in_maps, list(range(N_CORES))).results
    return np.stack([res[b]["out"] for b in range(N_CORES)], axis=0)
